# revision 2
# baseline (speedup 1.0000x reference)
"""Causal self-attention (softmax over the QUERY axis) for Trainium2, 8 cores.

Reference semantics (B=2, S=2048, D=1024, H=16, HD=64):
    q = x @ Wq; k = x @ Wk; v = x @ Wv          (per batch)
    s[b,h,q,k] = <q_bqh, k_bkh>;  mask k > q -> -inf
    w = softmax(s / sqrt(1024), axis=q)          # normalize over QUERY axis
    ctx[b,q,h,:] = sum_k w[b,h,q,k] * v[b,k,h,:]

Sharding: core c handles batch b = c // 4 and head group g = c % 4
(4 heads: 4g..4g+3).  Per core everything is done in a transposed
score layout S^T[k, q], which makes the query-axis softmax a FREE-AXIS
reduction, and the 1/Z[k] normalizer folds into V rows (no per-element
divide): ctx[q,d] = sum_k exp(s)/Z[k] * v[k,d] = sum_k exp(s) * (v[k,d]/Z[k]).

v2 over the original baseline:
  - Q/K projections run as fp8e4m3 DoubleRow matmuls (2 contraction
    chunks per pass): host ships x/Wq/Wk pre-quantized in the
    [128, c2, 2, n] DR layout.
  - Score rows for the two heads of a pair are emitted interleaved:
    head even lives in SBUF partitions 0-63 (PE row-tile T0), head odd
    in 64-127 (T8), so their matmul streams execute concurrently.
  - Causal diag masking is a pre-exp DVE add of a -1e30 triangle const
    onto the PSUM block, which makes the ACT accum_out Z exact
    (no gpsimd gather / inv-subtract machinery).
  - Z: ACT accum_out on rows kt<8 (2 chunks), DVE post-exp row reduce
    for kt>=8.
  - V is scaled by 1/Z in place; output is written/DMA'd as bf16.
"""

import numpy as np
import ml_dtypes
from contextlib import ExitStack

import concourse.bass as bass
import concourse.tile as tile
from concourse import bacc, mybir
from concourse.bass_utils import run_bass_kernel_spmd

BF16 = mybir.dt.bfloat16
F32 = mybir.dt.float32
FP8 = mybir.dt.float8e4
DR = mybir.MatmulPerfMode.DoubleRow

B, S, D, H, HD = 2, 2048, 1024, 16, 64
NCORES = 8
HL = 4                       # heads per core
KC = D // 128                # 8 contraction chunks (bf16)
C2 = D // 256                # 4 DR contraction chunks
KT = S // 128                # 16 key tiles
SCALE = 1.0 / float(np.sqrt(np.float32(D)))   # 1/32
NEG = -1.0e30


def _emit(ctx: ExitStack, tc: tile.TileContext, out_ap, xT, x8, w8q, w8k, wv,
          tri):
    nc = tc.nc
    Exp = mybir.ActivationFunctionType.Exp
    X = mybir.AxisListType.X
    ADD = mybir.AluOpType.add
    MULT = mybir.AluOpType.mult

    consts = ctx.enter_context(tc.tile_pool(name="consts", bufs=1))
    qkp = ctx.enter_context(tc.tile_pool(name="qk", bufs=1))
    vp = ctx.enter_context(tc.tile_pool(name="v", bufs=1))
    epool = ctx.enter_context(tc.tile_pool(name="e", bufs=2))
    zpool = ctx.enter_context(tc.tile_pool(name="z", bufs=4))
    outp = ctx.enter_context(tc.tile_pool(name="outp", bufs=1))
    # PSUM: scores 3 x [128,1024] (6 banks) + small 2 x [128,512] (2 banks)
    sc_ps = ctx.enter_context(tc.tile_pool(name="sc_ps", bufs=3, space="PSUM"))
    small_ps = ctx.enter_context(tc.tile_pool(name="small_ps", bufs=2,
                                              space="PSUM"))

    # ---- loads: small consts + xT on the SP HWDGE ring, x8 via gpsimd ----
    w8q_sb = consts.tile([128, C2, 2, HL * HD], FP8, tag="w8q", name="w8q_sb")
    w8k_sb = consts.tile([128, C2, 2, HL * HD], FP8, tag="w8k", name="w8k_sb")
    wv_sb = consts.tile([128, KC, HL * HD], BF16, tag="wv", name="wv_sb")
    tri_sb = consts.tile([128, 128], BF16, tag="tri", name="tri_sb")
    nc.sync.dma_start(out=w8q_sb, in_=w8q)
    nc.sync.dma_start(out=w8k_sb, in_=w8k)
    nc.sync.dma_start(out=tri_sb, in_=tri)
    nc.sync.dma_start(out=wv_sb, in_=wv.rearrange("(c p) n -> p c n", p=128))

    x8_cs = [None] * 4
    for sc in (3, 2, 1, 0):
        x8_cs[sc] = consts.tile([128, C2, 2, 512], FP8, tag=f"x8{sc}",
                                name=f"x8{sc}_sb")
        nc.gpsimd.dma_start(out=x8_cs[sc],
                            in_=x8[:, :, :, 512 * sc:512 * sc + 512])
    xT_r = xT.rearrange("(c p) s -> p c s", p=128)
    xT_cs = [None] * 4
    for sc in (3, 2, 1, 0):
        xT_cs[sc] = consts.tile([128, KC, 512], BF16, tag=f"xT{sc}",
                                name=f"xT{sc}_sb")
        nc.sync.dma_start(out=xT_cs[sc],
                          in_=xT_r[:, :, 512 * sc:512 * sc + 512])

    def xT_slice(c, lo, w):
        sc, o = divmod(lo, 512)
        assert o + w <= 512
        return xT_cs[sc][:, c, o:o + w]

    qT_sb = qkp.tile([128, 2, S], BF16, tag="qT")
    kT_sb = qkp.tile([128, 2, S], BF16, tag="kT")
    v_sb = vp.tile([128, KT, HL * HD], BF16, tag="v")
    out_sb = outp.tile([128, 2, S], BF16, tag="out")

    def proj_dr(name, pair, qc):
        """fp8 DoubleRow q/k projection for 512 s-cols."""
        w8 = w8q_sb if name == "q" else w8k_sb
        dst = qT_sb if name == "q" else kT_sb
        ps = small_ps.tile([128, 512], F32, tag="ps512", name="pps")
        for sub in (0, 1):
            for c2 in range(C2):
                nc.tensor.matmul(
                    ps[:, 256 * sub:256 * sub + 256],
                    w8[:, c2, :, 128 * pair:128 * pair + 128],
                    x8_cs[qc][:, c2, :, 256 * sub:256 * sub + 256],
                    start=(c2 == 0), stop=(c2 == C2 - 1),
                    perf_mode=DR,
                )
        nc.vector.tensor_copy(dst[:, pair, 512 * qc:512 * qc + 512], ps)

    def proj_v():
        # v natural layout: out partitions = s-within-tile, cols = 4 heads x 64
        for st in range(KT):
            ps = small_ps.tile([128, HL * HD], F32, tag="ps512", name="pps")
            for c in range(KC):
                nc.tensor.matmul(
                    ps,
                    xT_slice(c, 128 * st, 128),
                    wv_sb[:, c, :],
                    start=(c == 0), stop=(c == KC - 1),
                )
            nc.vector.tensor_copy(v_sb[:, st, :], ps)

    def alloc_head(h):
        zp = zpool.tile([128, KT, 2], F32, tag="zp", name=f"zp{h}")
        nc.vector.memset(zp, 0.0)
        return {"zp": zp, "e": [None] * KT, "h": h}

    def score_row_pair(sta, stb, kt):
        """scores^T row kt for a head pair, interleaved on PE tiles T0/T8."""
        pair = sta["h"] // 2
        q0k = 128 * kt
        W = S - q0k
        rows = {}
        for half, st in ((0, sta), (1, stb)):
            e_row = epool.tile([128, W], BF16, tag=f"E{kt}h{half}",
                               name=f"e{kt}h{half}",
                               bufs=(2 if kt >= 4 else 1))
            st["e"][kt] = e_row
            rows[half] = e_row
        chunks = []
        lo = 0
        while lo < W:
            w = min(1024, W - lo)
            chunks.append((lo, w))
            lo += w
        for ci, (lo, w) in enumerate(chunks):
            pss = {}
            for half in (0, 1):
                pss[half] = sc_ps.tile([128, w], F32, tag="sc",
                                       name=f"sc{kt}h{half}")
            c0 = 0
            while c0 < w:
                c1 = min(w, c0 + 512)
                for half in (0, 1):
                    pb = 64 * half
                    nc.tensor.matmul(
                        pss[half][:, c0:c1],
                        kT_sb[pb:pb + 64, pair, q0k:q0k + 128],
                        qT_sb[pb:pb + 64, pair, q0k + lo + c0:q0k + lo + c1],
                        start=True, stop=True,
                    )
                c0 = c1
            if ci == 0:
                # causal diag: cols [0,128) hold q in [q0k, q0k+128);
                # entries with q < k (j < p) get -1e30 before exp
                for half in (0, 1):
                    nc.vector.scalar_tensor_tensor(
                        out=pss[half][:, 0:128], in0=pss[half][:, 0:128],
                        scalar=1.0, in1=tri_sb, op0=MULT, op1=ADD,
                    )
            for half, st in ((0, sta), (1, stb)):
                if kt < 8:
                    nc.scalar.activation(
                        rows[half][:, lo:lo + w], pss[half][:, 0:w],
                        Exp, scale=SCALE,
                        accum_out=st["zp"][:, kt, ci:ci + 1],
                    )
                else:
                    nc.scalar.activation(
                        rows[half][:, lo:lo + w], pss[half][:, 0:w],
                        Exp, scale=SCALE,
                    )
        if kt >= 8:
            for half, st in ((0, sta), (1, stb)):
                nc.vector.tensor_reduce(
                    st["zp"][:, kt, 0:1], rows[half][:, 0:W],
                    axis=X, op=ADD,
                )

    def z_v2(st):
        """finalize Z and scale V rows by 1/Z in place (head slice)."""
        h = st["h"]
        zs = zpool.tile([128, KT], F32, tag="zs", name="zs")
        nc.vector.tensor_reduce(zs, st["zp"], axis=X, op=ADD)
        zi = zpool.tile([128, KT], F32, tag="zi", name="zi")
        nc.vector.reciprocal(zi, zs)
        zia = zi[:, :]
        zi_bc = bass.AP(tensor=zia.tensor, offset=zia.offset,
                        ap=[zia.ap[0], zia.ap[1], [0, HD]])
        nc.vector.tensor_mul(
            v_sb[:, :, HD * h:HD * h + HD],
            v_sb[:, :, HD * h:HD * h + HD],
            zi_bc,
        )

    def ctx_pair_packed(sta, stb, qc):
        """col-packed ctx chains for a whole pair (heads sta, stb) at qc."""
        pair = sta["h"] // 2
        ps = small_ps.tile([128, 512], F32, tag="ps512", name="cpp")
        n_kt = 4 * qc + 4
        for kt in range(n_kt):
            q0 = max(512 * qc, 128 * kt)
            w = 512 * qc + 512 - q0
            for half, st in ((0, sta), (1, stb)):
                h = st["h"]
                rhs = st["e"][kt][:, q0 - 128 * kt:q0 - 128 * kt + w]
                nc.tensor.matmul(
                    ps[64 * half:64 * half + 64, q0 - 512 * qc:512],
                    v_sb[:, kt, HD * h:HD * h + HD],
                    rhs,
                    start=(kt == 0), stop=(kt == n_kt - 1),
                    tile_position=(0, 64 * half),
                    skip_group_check=True,
                )
        nc.vector.tensor_copy(out_sb[:, pair, 512 * qc:512 * qc + 512], ps)

    def out_dma(pair, qc):
        nc.sync.dma_start(
            out=out_ap[128 * pair:128 * pair + 128, 512 * qc:512 * qc + 512],
            in_=out_sb[:, pair, 512 * qc:512 * qc + 512],
        )

    # ---- emission (order = scheduling priority) ----
    st0, st1 = alloc_head(0), alloc_head(1)
    for qc in (3, 2, 1, 0):
        proj_dr("q", 0, qc)
        proj_dr("k", 0, qc)
        for kt in range(4 * qc + 3, 4 * qc - 1, -1):
            score_row_pair(st0, st1, kt)
    st2, st3 = alloc_head(2), alloc_head(3)
    for qc in (3, 2):
        proj_dr("q", 1, qc)
        proj_dr("k", 1, qc)
        for kt in range(4 * qc + 3, 4 * qc - 1, -1):
            score_row_pair(st2, st3, kt)
    proj_v()
    z_v2(st0)
    z_v2(st1)
    proj_dr("q", 1, 1)
    proj_dr("k", 1, 1)
    for kt in (7, 6, 5, 4):
        score_row_pair(st2, st3, kt)
    ctx_pair_packed(st0, st1, 0)
    out_dma(0, 0)
    ctx_pair_packed(st0, st1, 1)
    out_dma(0, 1)
    ctx_pair_packed(st0, st1, 2)
    ctx_pair_packed(st0, st1, 3)   # frees all pair-0 E rows (incl. 0..3)
    out_dma(0, 2)
    out_dma(0, 3)
    proj_dr("q", 1, 0)
    proj_dr("k", 1, 0)
    for kt in (3, 2, 1, 0):
        score_row_pair(st2, st3, kt)
    z_v2(st2)
    z_v2(st3)
    for g in range(4):
        ctx_pair_packed(st2, st3, g)
        out_dma(1, g)


_PROG = None


def _build_program():
    global _PROG
    if _PROG is not None:
        return _PROG
    nc = bacc.Bacc("TRN2", target_bir_lowering=False, debug=False,
                   num_devices=NCORES)
    xT = nc.dram_tensor("xT", [D, S], BF16, kind="ExternalInput").ap()
    x8 = nc.dram_tensor("x8", [128, C2, 2, S], FP8, kind="ExternalInput").ap()
    w8q = nc.dram_tensor("w8q", [128, C2, 2, HL * HD], FP8,
                         kind="ExternalInput").ap()
    w8k = nc.dram_tensor("w8k", [128, C2, 2, HL * HD], FP8,
                         kind="ExternalInput").ap()
    wv = nc.dram_tensor("wv", [D, HL * HD], BF16, kind="ExternalInput").ap()
    tri = nc.dram_tensor("tri", [128, 128], BF16, kind="ExternalInput").ap()
    out = nc.dram_tensor("out", [HL * HD, S], BF16, kind="ExternalOutput").ap()
    with tile.TileContext(nc) as tc:
        with ExitStack() as stack:
            _emit(stack, tc, out, xT, x8, w8q, w8k, wv, tri)
    nc.compile()
    _PROG = nc
    return nc


def _dr_layout(a):
    """[1024, n] -> [128, 4, 2, n] with row r = 256*c2 + 128*i + p."""
    n = a.shape[1]
    return np.ascontiguousarray(
        a.reshape(C2, 2, 128, n).transpose(2, 0, 1, 3))


def make_in_maps(x, Wq, Wk, Wv):
    bf = ml_dtypes.bfloat16
    f8 = ml_dtypes.float8_e4m3fn
    tri = np.tril(np.full((128, 128), NEG, np.float32), -1).astype(bf)
    in_maps = []
    for core in range(NCORES):
        b, g = divmod(core, NCORES // B)
        cols = slice(HL * HD * g, HL * HD * (g + 1))
        xb = np.ascontiguousarray(np.asarray(x[b]).T)
        in_maps.append({
            "xT": xb.astype(bf),
            "x8": _dr_layout(xb).astype(f8),
            "w8q": _dr_layout(np.asarray(Wq)[:, cols]).astype(f8),
            "w8k": _dr_layout(np.asarray(Wk)[:, cols]).astype(f8),
            "wv": np.ascontiguousarray(np.asarray(Wv)[:, cols]).astype(bf),
            "tri": tri,
        })
    return in_maps


def assemble(results):
    out = np.empty((B, S, H * HD), np.float32)
    for core in range(NCORES):
        b, g = divmod(core, NCORES // B)
        out[b, :, HL * HD * g:HL * HD * (g + 1)] = \
            results[core]["out"].astype(np.float32).T
    return out


def kernel(**inputs):
    nc = _build_program()
    in_maps = make_in_maps(inputs["x"], inputs["Wq"], inputs["Wk"], inputs["Wv"])
    res = run_bass_kernel_spmd(nc, in_maps, list(range(NCORES)))
    return assemble(res.results)


# revision 4
# speedup vs baseline: 1.1275x; 1.1275x over previous
"""Causal self-attention (softmax over the QUERY axis) for Trainium2, 8 cores.

Reference semantics (B=2, S=2048, D=1024, H=16, HD=64):
    q = x @ Wq; k = x @ Wk; v = x @ Wv          (per batch)
    s[b,h,q,k] = <q_bqh, k_bkh>;  mask k > q -> -inf
    w = softmax(s / sqrt(1024), axis=q)          # normalize over QUERY axis
    ctx[b,q,h,:] = sum_k w[b,h,q,k] * v[b,k,h,:]

Sharding: core c handles batch b = c // 4 and head group g = c % 4
(4 heads: 4g..4g+3).  Per core everything is done in a transposed
score layout S^T[k, q], which makes the query-axis softmax a FREE-AXIS
reduction, and the 1/Z[k] normalizer folds into V rows (no per-element
divide): ctx[q,d] = sum_k exp(s)/Z[k] * v[k,d] = sum_k exp(s) * (v[k,d]/Z[k]).

v3 structure:
  - Score rows for the two heads of a pair are emitted interleaved per
    512-col subchunk: head even lives in SBUF partitions 0-63 (PE row
    tile T0), head odd in 64-127 (T8), with separate PSUM pools, so the
    two matmul streams execute concurrently on the tiled PE array.
  - Causal diag handling: rows kt<8 get a pre-exp DVE add of a -1e30
    triangle const onto the PSUM diag block (makes ACT accum_out Z
    exact); rows kt>=8 get the baseline post-exp gpsimd affine_select.
  - Z: ACT accum_out for kt<8, DVE row reduce for kt 8..11, gpsimd row
    reduce for kt 12..15.
  - V is scaled by 1/Z in place; output is written/DMA'd as bf16.
"""

import numpy as np
import ml_dtypes
from contextlib import ExitStack

import concourse.bass as bass
import concourse.tile as tile
from concourse import bacc, mybir
from concourse.bass_utils import run_bass_kernel_spmd

BF16 = mybir.dt.bfloat16
F32 = mybir.dt.float32

B, S, D, H, HD = 2, 2048, 1024, 16, 64
NCORES = 8
HL = 4                       # heads per core
KC = D // 128                # 8 contraction chunks
KT = S // 128                # 16 key tiles
SCALE = 1.0 / float(np.sqrt(np.float32(D)))   # 1/32
NEG = -1.0e30


def _emit(ctx: ExitStack, tc: tile.TileContext, out_ap, xT, wq, wk, wv, tri):
    nc = tc.nc
    Exp = mybir.ActivationFunctionType.Exp
    X = mybir.AxisListType.X
    ADD = mybir.AluOpType.add
    MULT = mybir.AluOpType.mult

    consts = ctx.enter_context(tc.tile_pool(name="consts", bufs=1))
    qkp = ctx.enter_context(tc.tile_pool(name="qk", bufs=1))
    vp = ctx.enter_context(tc.tile_pool(name="v", bufs=1))
    epool = ctx.enter_context(tc.tile_pool(name="e", bufs=2))
    zpool = ctx.enter_context(tc.tile_pool(name="z", bufs=4))
    outp = ctx.enter_context(tc.tile_pool(name="outp", bufs=1))
    # PSUM: scores 2 x [128,1536] (A/B, 6 banks) + small 2 x [128,512]
    scA_ps = ctx.enter_context(tc.tile_pool(name="scA_ps", bufs=1,
                                            space="PSUM"))
    scB_ps = ctx.enter_context(tc.tile_pool(name="scB_ps", bufs=1,
                                            space="PSUM"))
    small_ps = ctx.enter_context(tc.tile_pool(name="small_ps", bufs=2,
                                              space="PSUM"))

    # ---- loads (all on the SP HWDGE ring; score rows descend, so xT
    # chunk 3 first) ----
    w_sb = {}
    for name, t in (("q", wq), ("k", wk), ("v", wv)):
        w_sb[name] = consts.tile([128, KC, HL * HD], BF16, tag=f"w{name}",
                                 name=f"w{name}_sb")
        nc.sync.dma_start(out=w_sb[name], in_=t.rearrange("(c p) n -> p c n",
                                                          p=128))
    tri_sb = consts.tile([128, 128], BF16, tag="tri", name="tri_sb")
    nc.sync.dma_start(out=tri_sb, in_=tri)
    xT_r = xT.rearrange("(c p) s -> p c s", p=128)
    xT_cs = [None] * 4
    for sc in (3, 2, 1, 0):
        xT_cs[sc] = consts.tile([128, KC, 512], BF16, tag=f"xT{sc}",
                                name=f"xT{sc}_sb")
        nc.scalar.dma_start(out=xT_cs[sc],
                            in_=xT_r[:, :, 512 * sc:512 * sc + 512])

    def xT_slice(c, lo, w):
        sc, o = divmod(lo, 512)
        assert o + w <= 512
        return xT_cs[sc][:, c, o:o + w]

    qT_sb = qkp.tile([128, 2, S], BF16, tag="qT")
    kT_sb = qkp.tile([128, 2, S], BF16, tag="kT")
    v_sb = vp.tile([128, KT, HL * HD], BF16, tag="v")
    out_sb = outp.tile([128, 2, S], BF16, tag="out")

    def proj_chain(name, pair, qc):
        dst = qT_sb if name == "q" else kT_sb
        ps = small_ps.tile([128, 512], F32, tag="ps512", name="pps")
        for c in range(KC):
            nc.tensor.matmul(
                ps,
                w_sb[name][:, c, 128 * pair:128 * pair + 128],
                xT_cs[qc][:, c, :],
                start=(c == 0), stop=(c == KC - 1),
            )
        nc.vector.tensor_copy(dst[:, pair, 512 * qc:512 * qc + 512], ps)

    def proj_v():
        # v natural layout: out partitions = s-within-tile, cols = 4 heads x 64
        for st in range(KT):
            ps = small_ps.tile([128, HL * HD], F32, tag="ps512", name="pps")
            for c in range(KC):
                nc.tensor.matmul(
                    ps,
                    xT_slice(c, 128 * st, 128),
                    w_sb["v"][:, c, :],
                    start=(c == 0), stop=(c == KC - 1),
                )
            nc.vector.tensor_copy(v_sb[:, st, :], ps)

    def alloc_head(h):
        zp = zpool.tile([128, KT, 2], F32, tag="zp", name=f"zp{h}")
        nc.vector.memset(zp, 0.0)
        return {"zp": zp, "e": [None] * KT, "h": h}

    def score_row_pair(sta, stb, kt):
        """scores^T row kt for a head pair, interleaved on PE tiles T0/T8."""
        pair = sta["h"] // 2
        q0k = 128 * kt
        W = S - q0k
        rows = {}
        for half, st in ((0, sta), (1, stb)):
            e_row = epool.tile([128, W], BF16, tag=f"E{kt}h{half}",
                               name=f"e{kt}h{half}",
                               bufs=(2 if kt >= 4 else 1))
            st["e"][kt] = e_row
            rows[half] = e_row
        chunks = [(0, min(W, 1536))]
        if W > 1536:
            chunks.append((1536, W - 1536))
        for ci, (lo, w) in enumerate(chunks):
            pss = {0: scA_ps.tile([128, w], F32, tag="sc", name=f"sA{kt}"),
                   1: scB_ps.tile([128, w], F32, tag="sc", name=f"sB{kt}")}
            c0 = 0
            while c0 < w:
                c1 = min(w, c0 + 512)
                for half in (0, 1):
                    pb = 64 * half
                    nc.tensor.matmul(
                        pss[half][:, c0:c1],
                        kT_sb[pb:pb + 64, pair, q0k:q0k + 128],
                        qT_sb[pb:pb + 64, pair, q0k + lo + c0:q0k + lo + c1],
                        start=True, stop=True,
                    )
                c0 = c1
            if ci == 0 and kt < 8:
                # pre-exp diag mask -> accum_out Z is exact
                for half in (0, 1):
                    nc.vector.scalar_tensor_tensor(
                        out=pss[half][:, 0:128], in0=pss[half][:, 0:128],
                        scalar=1.0, in1=tri_sb, op0=MULT, op1=ADD,
                    )
            for half, st in ((0, sta), (1, stb)):
                if kt < 8:
                    nc.scalar.activation(
                        rows[half][:, lo:lo + w], pss[half][:, 0:w],
                        Exp, scale=SCALE,
                        accum_out=st["zp"][:, kt, ci:ci + 1],
                    )
                else:
                    nc.scalar.activation(
                        rows[half][:, lo:lo + w], pss[half][:, 0:w],
                        Exp, scale=SCALE,
                    )
        if kt >= 8:
            # post-exp diag mask on gpsimd (j < p  <=>  p - j - 1 >= 0
            # negated: keep j >= p), then row-sum Z off the ACT engine
            for half, st in ((0, sta), (1, stb)):
                diag = rows[half][:, 0:128]
                nc.gpsimd.affine_select(
                    diag, diag, pattern=[[1, 128]],
                    compare_op=mybir.AluOpType.is_ge, fill=0.0,
                    base=0, channel_multiplier=-1,
                )
                nc.vector.tensor_reduce(
                    st["zp"][:, kt, 0:1], rows[half][:, 0:W],
                    axis=X, op=ADD,
                )

    def z_v2(st):
        """finalize Z and scale V rows by 1/Z in place (head slice)."""
        h = st["h"]
        zs = zpool.tile([128, KT], F32, tag="zs", name="zs")
        nc.vector.tensor_reduce(zs, st["zp"], axis=X, op=ADD)
        zi = zpool.tile([128, KT], F32, tag="zi", name="zi")
        nc.vector.reciprocal(zi, zs)
        zia = zi[:, :]
        zi_bc = bass.AP(tensor=zia.tensor, offset=zia.offset,
                        ap=[zia.ap[0], zia.ap[1], [0, HD]])
        nc.vector.tensor_mul(
            v_sb[:, :, HD * h:HD * h + HD],
            v_sb[:, :, HD * h:HD * h + HD],
            zi_bc,
        )

    def ctx_pair_packed(sta, stb, qc):
        """col-packed ctx chains for a whole pair (heads sta, stb) at qc."""
        pair = sta["h"] // 2
        ps = small_ps.tile([128, 512], F32, tag="ps512", name="cpp")
        n_kt = 4 * qc + 4
        for kt in range(n_kt):
            q0 = max(512 * qc, 128 * kt)
            w = 512 * qc + 512 - q0
            for half, st in ((0, sta), (1, stb)):
                h = st["h"]
                rhs = st["e"][kt][:, q0 - 128 * kt:q0 - 128 * kt + w]
                nc.tensor.matmul(
                    ps[64 * half:64 * half + 64, q0 - 512 * qc:512],
                    v_sb[:, kt, HD * h:HD * h + HD],
                    rhs,
                    start=(kt == 0), stop=(kt == n_kt - 1),
                    tile_position=(0, 64 * half),
                    skip_group_check=True,
                )
        nc.vector.tensor_copy(out_sb[:, pair, 512 * qc:512 * qc + 512], ps)

    def out_dma(pair, qc):
        nc.sync.dma_start(
            out=out_ap[128 * pair:128 * pair + 128, 512 * qc:512 * qc + 512],
            in_=out_sb[:, pair, 512 * qc:512 * qc + 512],
        )

    # ---- emission (order = scheduling priority) ----
    st0, st1 = alloc_head(0), alloc_head(1)
    for qc in (3, 2, 1, 0):
        proj_chain("q", 0, qc)
        proj_chain("k", 0, qc)
        for kt in range(4 * qc + 3, 4 * qc - 1, -1):
            score_row_pair(st0, st1, kt)
    st2, st3 = alloc_head(2), alloc_head(3)
    for qc in (3, 2):
        proj_chain("q", 1, qc)
        proj_chain("k", 1, qc)
        for kt in range(4 * qc + 3, 4 * qc - 1, -1):
            score_row_pair(st2, st3, kt)
    proj_v()
    z_v2(st0)
    z_v2(st1)
    proj_chain("q", 1, 1)
    proj_chain("k", 1, 1)
    for kt in (7, 6, 5, 4):
        score_row_pair(st2, st3, kt)
    ctx_pair_packed(st0, st1, 0)
    out_dma(0, 0)
    ctx_pair_packed(st0, st1, 1)
    out_dma(0, 1)
    ctx_pair_packed(st0, st1, 2)
    ctx_pair_packed(st0, st1, 3)   # frees all pair-0 E rows (incl. 0..3)
    out_dma(0, 2)
    out_dma(0, 3)
    proj_chain("q", 1, 0)
    proj_chain("k", 1, 0)
    for kt in (3, 2, 1, 0):
        score_row_pair(st2, st3, kt)
    z_v2(st2)
    z_v2(st3)
    for g in range(4):
        ctx_pair_packed(st2, st3, g)
        out_dma(1, g)


_PROG = None


def _build_program():
    global _PROG
    if _PROG is not None:
        return _PROG
    nc = bacc.Bacc("TRN2", target_bir_lowering=False, debug=False,
                   num_devices=NCORES)
    xT = nc.dram_tensor("xT", [D, S], BF16, kind="ExternalInput").ap()
    wq = nc.dram_tensor("wq", [D, HL * HD], BF16, kind="ExternalInput").ap()
    wk = nc.dram_tensor("wk", [D, HL * HD], BF16, kind="ExternalInput").ap()
    wv = nc.dram_tensor("wv", [D, HL * HD], BF16, kind="ExternalInput").ap()
    tri = nc.dram_tensor("tri", [128, 128], BF16, kind="ExternalInput").ap()
    out = nc.dram_tensor("out", [HL * HD, S], BF16, kind="ExternalOutput").ap()
    with tile.TileContext(nc) as tc:
        with ExitStack() as stack:
            _emit(stack, tc, out, xT, wq, wk, wv, tri)
    nc.compile()
    _PROG = nc
    return nc


def make_in_maps(x, Wq, Wk, Wv):
    bf = ml_dtypes.bfloat16
    tri = np.tril(np.full((128, 128), NEG, np.float32), -1).astype(bf)
    in_maps = []
    for core in range(NCORES):
        b, g = divmod(core, NCORES // B)
        cols = slice(HL * HD * g, HL * HD * (g + 1))
        in_maps.append({
            "xT": np.ascontiguousarray(np.asarray(x[b]).T).astype(bf),
            "wq": np.ascontiguousarray(np.asarray(Wq)[:, cols]).astype(bf),
            "wk": np.ascontiguousarray(np.asarray(Wk)[:, cols]).astype(bf),
            "wv": np.ascontiguousarray(np.asarray(Wv)[:, cols]).astype(bf),
            "tri": tri,
        })
    return in_maps


def assemble(results):
    out = np.empty((B, S, H * HD), np.float32)
    for core in range(NCORES):
        b, g = divmod(core, NCORES // B)
        out[b, :, HL * HD * g:HL * HD * (g + 1)] = \
            results[core]["out"].astype(np.float32).T
    return out


def kernel(**inputs):
    nc = _build_program()
    in_maps = make_in_maps(inputs["x"], inputs["Wq"], inputs["Wk"], inputs["Wv"])
    res = run_bass_kernel_spmd(nc, in_maps, list(range(NCORES)))
    return assemble(res.results)
